# revision 35
# baseline (speedup 1.0000x reference)
"""BigBird sparse attention kernel for 8 Trainium2 NeuronCores.

Sharding: token-parallel. B=2 batches x 4 chunks of 1024 local tokens each
-> 8 cores. Each core receives a transposed x-slice [D=1024, 1282] whose
columns are [g0, g1, 10 window blocks of 128 tokens] (blocks 8j-1 .. 8j+8,
zero-padded outside [0, 32)). The core computes:
  - q/k projections in transposed layout [f, tok] (fp32r matmuls)
  - v projection in [tok, f] layout
  - 3-block sliding-window attention with scores kept transposed [kt, q]
    (exp'd probabilities feed P.V matmuls directly, denominator rides as a
    ones column in the V stationary)
  - attention of local tokens to the 2 global tokens (separate softmax)
  - flash-style partial stats (sum-exp, weighted V) of the 2 global query
    tokens against the core's local keys -> combined on host
  - output projection + bias for its 1024 local tokens
Host assembles the 8 slices, and computes the 2 global output rows per
batch exactly in numpy from the shipped partials.
"""

import numpy as np
import ml_dtypes

import concourse.bass as bass
import concourse.mybir as mybir
import concourse.tile as tile
from concourse import bacc
from concourse.bass_utils import run_bass_kernel_spmd

F32 = mybir.dt.float32
F32R = mybir.dt.float32r
BF16 = mybir.dt.bfloat16
AF = mybir.ActivationFunctionType
BF = ml_dtypes.bfloat16

D_MODEL = 1024
H = 16
DK = 64
BS = 128
B = 2
T = 4098
NB = 32            # global 128-blocks of local tokens
NW = 10            # window blocks per core (8 local + 2 halo)
TOKS = 2 + NW * BS # x-slice columns
LQ0 = 2 + BS       # first local-q column
SCALE = 1.0 / np.sqrt(DK)

# token chunks for the projection moving dim (all >=256 for fp32r speed)
CHUNKS = [(0, 512), (512, 512), (1024, 258)]


def C(t):
    return 2 + BS * t


# P.V accumulation schedule per psum bank: (t, qstart, nblocks, start, stop)
# bank 0 covers q window-positions 1..4, bank 1 covers 5..8.
PV_SCHED = [
    [(2, 1, 3, True, False), (3, 2, 2, False, False), (3, 4, 1, False, False),
     (0, 1, 1, False, False), (1, 1, 2, False, False), (4, 3, 2, False, False),
     (5, 4, 1, False, True)],
    [(6, 5, 3, True, False), (7, 6, 2, False, False), (7, 8, 1, False, False),
     (4, 5, 1, False, False), (5, 5, 2, False, False), (8, 7, 2, False, False),
     (9, 8, 1, False, True)],
]


def ptcol(t, qpos):
    # column of (window-block t, q window-position qpos) in the pt tensor
    return 384 * t + 128 * (qpos - (t - 1))


def build_kernel(nc):
    xt = nc.dram_tensor("xt", [D_MODEL, TOKS], F32, kind="ExternalInput").ap()
    wq = nc.dram_tensor("wq", [8, 8, 128, 128], F32, kind="ExternalInput").ap()
    wk = nc.dram_tensor("wk", [8, 8, 128, 128], F32, kind="ExternalInput").ap()
    wv = nc.dram_tensor("wv", [2, 8, 128, 512], F32, kind="ExternalInput").ap()
    wo = nc.dram_tensor("wo", [8, 8, 128, 128], BF16, kind="ExternalInput").ap()
    bo = nc.dram_tensor("bo", [D_MODEL], F32, kind="ExternalInput").ap()
    maskl = nc.dram_tensor("maskl", [128, 1], BF16, kind="ExternalInput").ap()
    maskr = nc.dram_tensor("maskr", [128, 1], BF16, kind="ExternalInput").ap()
    outt = nc.dram_tensor("outt", [D_MODEL, 1024], F32, kind="ExternalOutput").ap()
    gstats = nc.dram_tensor("gstats", [65, 32], F32, kind="ExternalOutput").ap()
    import os as _os
    dbg = None
    if _os.environ.get("BB_DEBUG"):
        dbg = nc.dram_tensor("dbg_at", [D_MODEL, 1024], BF16,
                             kind="ExternalOutput").ap()

    with tile.TileContext(nc) as tc:
        with (
            tc.tile_pool(name="pc", bufs=1) as pc,
            tc.tile_pool(name="px", bufs=1) as px,
            tc.tile_pool(name="pqk", bufs=1) as pqk,
            tc.tile_pool(name="pv", bufs=1) as pvp,
            tc.tile_pool(name="pwv", bufs=1) as pwv,
            tc.tile_pool(name="pw", bufs=6) as pw,
            tc.tile_pool(name="pat", bufs=1) as pat,
            tc.tile_pool(name="ppt", bufs=2) as ppt,
            tc.tile_pool(name="psm", bufs=2) as psm,
            tc.tile_pool(name="pout", bufs=2) as pout,
            tc.tile_pool(name="pps", bufs=8, space="PSUM") as pps,
        ):
            # ---- constants ----
            bo_sb = pc.tile([128, 8], F32, tag="bo")
            nc.sync.dma_start(bo_sb[:], bo.rearrange("(t p) -> p t", p=128))
            ml_sb = pc.tile([128, 1], BF16, tag="ml")
            mr_sb = pc.tile([128, 1], BF16, tag="mr")
            nc.sync.dma_start(ml_sb[:], maskl)
            nc.sync.dma_start(mr_sb[:], maskr)
            ones32 = pc.tile([2, 32], BF16, tag="ones32")
            nc.vector.memset(ones32[:], 0.0)
            nc.vector.memset(ones32[:, 0:1], 1.0)
            gst = pc.tile([65, 32], F32, tag="gst")

            # ---- x slice, transposed, resident ----
            xts = []
            for d in range(8):
                xd = px.tile([128, TOKS], F32R, tag=f"xt{d}")
                rows = xt[128 * d:128 * (d + 1), :].bitcast(F32R)
                nc.sync.dma_start(xd[:, 0:512], rows[:, 0:512])
                nc.sync.dma_start(xd[:, 512:TOKS], rows[:, 512:TOKS])
                xts.append(xd)

            at_sb = [pat.tile([128, 1024], BF16, tag=f"at{f}", name=f"at{f}")
                     for f in range(8)]

            def emit_qk_proj(pss, half, qk_tiles):
                for pname, wdram in (("q", wq), ("k", wk)):
                    osbs, psjs = [], []
                    for i2 in range(2):
                        i = 2 * half + i2
                        osb = pqk.tile([128, TOKS], F32R, tag=f"qk{pname}{i}",
                                       name=f"qk{pname}{i}")
                        qk_tiles[(pname, i)] = osb
                        osbs.append(osb)
                        psjs.append([pps.tile([128, cn], F32, tag="ps1",
                                              name=f"pj{i2}_{c}")
                                     for c, (c0, cn) in enumerate(CHUNKS)])
                    for d in range(8):
                        wt = pw.tile([128, 256], F32R, tag="w")
                        ft0 = 4 * pss + 2 * half
                        src = bass.AP(wdram.tensor,
                                      wdram[ft0, d].offset,
                                      [[128, 128], [8 * 128 * 128, 2], [1, 128]])
                        nc.sync.dma_start(wt[:], src.bitcast(F32R))
                        for i2 in range(2):
                            for c, (c0, cn) in enumerate(CHUNKS):
                                nc.tensor.matmul(
                                    psjs[i2][c][:, :cn],
                                    wt[:, 128 * i2:128 * i2 + 128],
                                    xts[d][:, c0:c0 + cn],
                                    start=(d == 0), stop=(d == 7))
                    for i2 in range(2):
                        for c, (c0, cn) in enumerate(CHUNKS):
                            if pname == "q":
                                nc.scalar.mul(osbs[i2][:, c0:c0 + cn],
                                              psjs[i2][c][:, :cn], SCALE)
                            else:
                                nc.scalar.copy(osbs[i2][:, c0:c0 + cn],
                                               psjs[i2][c][:, :cn])

            def emit_v_proj(pss):
                wv_sb = []
                for d in range(8):
                    wvd = pwv.tile([128, 512], F32R, tag=f"wv{d}", name=f"wv{d}")
                    nc.sync.dma_start(wvd[:], wv[pss, d].bitcast(F32R))
                    wv_sb.append(wvd)
                v96 = []
                for tb in range(NW):
                    pv_ps = pps.tile([128, 512], F32, tag="ps1", name="pv_ps")
                    for d in range(8):
                        nc.tensor.matmul(pv_ps[:], xts[d][:, C(tb):C(tb) + 128],
                                         wv_sb[d][:], start=(d == 0), stop=(d == 7))
                    vt = pvp.tile([128, 8 * 96], BF16, tag=f"v96_{tb}",
                                  name=f"v96_{tb}")
                    pstep = vt.ap[0][0]
                    dst = bass.AP(vt.tensor, vt[:].offset,
                                  [[pstep, 128], [96, 8], [1, 64]])
                    src = bass.AP(pv_ps.tensor, pv_ps[:].offset,
                                  [[pv_ps.ap[0][0], 128], [64, 8], [1, 64]])
                    nc.vector.tensor_copy(dst, src)
                    onesap = bass.AP(vt.tensor, vt[:].offset + 64,
                                     [[pstep, 128], [96, 8], [1, 1]])
                    nc.vector.memset(onesap, 1.0)
                    zap = bass.AP(vt.tensor, vt[:].offset + 65,
                                  [[pstep, 128], [96, 8], [1, 31]])
                    nc.vector.memset(zap, 0.0)
                    v96.append(vt)
                pvg = pps.tile([2, 512], F32, tag="ps1", name="pvg")
                for d in range(8):
                    nc.tensor.matmul(pvg[:], xts[d][:, 0:2], wv_sb[d][:],
                                     start=(d == 0), stop=(d == 7))
                vg_sb = pvp.tile([2, 8 * 96], BF16, tag="vg", name="vg")
                gstep = vg_sb.ap[0][0]
                gdst = bass.AP(vg_sb.tensor, vg_sb[:].offset,
                               [[gstep, 2], [96, 8], [1, 64]])
                gsrc = bass.AP(pvg.tensor, pvg[:].offset,
                               [[pvg.ap[0][0], 2], [64, 8], [1, 64]])
                nc.scalar.copy(gdst, gsrc)
                g1 = bass.AP(vg_sb.tensor, vg_sb[:].offset + 64,
                             [[gstep, 2], [96, 8], [1, 1]])
                nc.vector.memset(g1, 1.0)
                g0 = bass.AP(vg_sb.tensor, vg_sb[:].offset + 65,
                             [[gstep, 2], [96, 8], [1, 31]])
                nc.vector.memset(g0, 0.0)
                return v96, vg_sb

            def emit_head(h, qk_tiles, v96, vg_sb):
                hl = h % 8
                r0 = 64 * (hl % 2)
                qh = qk_tiles[("q", hl // 2)][r0:r0 + 64, :]
                kh = qk_tiles[("k", hl // 2)][r0:r0 + 64, :]

                # xg scores + exp first so ACT serves them before the
                # score exps (oxg/wv matmuls then never wait on ACT backlog)
                pxg = psm.tile([2, 1024], BF16, tag="pxg", name="pxg", bufs=3)
                for c in range(2):
                    ps_xg = pps.tile([2, 512], F32, tag="ps1", name="ps_xg")
                    nc.tensor.matmul(ps_xg[:], kh[:, 0:2],
                                     qh[:, LQ0 + 512 * c:LQ0 + 512 * c + 512],
                                     start=True, stop=True)
                    nc.scalar.activation(pxg[:, 512 * c:512 * c + 512], ps_xg[:],
                                         AF.Exp)
                pt = ppt.tile([128, 3840], BF16, tag="pt", name="pt")
                psg = pps.tile([128, 16], F32, tag="ps1", name="psg")
                for t in range(1, 9):
                    nc.tensor.matmul(psg[:, 2 * (t - 1):2 * t],
                                     kh[:, C(t):C(t) + 128], qh[:, 0:2],
                                     start=(t == 1), stop=(t == 8))
                pg = psm.tile([128, 16], BF16, tag="pgsb", name="pg", bufs=3)
                nc.scalar.activation(pg[:], psg[:], AF.Exp)
                for t in range(NW):
                    qlo, qhi = max(t - 1, 1), min(t + 1, 8)
                    n = (qhi - qlo + 1) * 128
                    ps_s = pps.tile([128, 384], F32, tag="ps1", name="ps_s")
                    nc.tensor.matmul(ps_s[:, :n], kh[:, C(t):C(t) + 128],
                                     qh[:, C(qlo):C(qlo) + n],
                                     start=True, stop=True)
                    col = ptcol(t, qlo)
                    nc.scalar.activation(pt[:, col:col + n], ps_s[:, :n], AF.Exp)
                    if t == 0:
                        nc.gpsimd.tensor_mul(pt[:, col:col + n], pt[:, col:col + n],
                                             ml_sb[:].to_broadcast((128, n)))
                    if t == NW - 1:
                        nc.gpsimd.tensor_mul(pt[:, col:col + n], pt[:, col:col + n],
                                             mr_sb[:].to_broadcast((128, n)))
                ps_ob = [pps.tile([96, 512], F32, tag="ps1", name=f"po{bank}")
                         for bank in range(2)]
                for bank in range(2):
                    for (t, qs, nb, st, sp) in PV_SCHED[bank]:
                        c0 = 128 * (qs - 1) - 512 * bank
                        nc.tensor.matmul(
                            ps_ob[bank][:, c0:c0 + 128 * nb],
                            v96[t][:, 96 * hl:96 * hl + 96],
                            pt[:, ptcol(t, qs):ptcol(t, qs) + 128 * nb],
                            start=st, stop=sp)
                ps_wv = pps.tile([96, 2], F32, tag="ps1", name="ps_wv")
                for t in range(1, 9):
                    nc.tensor.matmul(ps_wv[:], v96[t][:, 96 * hl:96 * hl + 96],
                                     pg[:, 2 * (t - 1):2 * t],
                                     start=(t == 1), stop=(t == 8))
                ps_oxb = [pps.tile([96, 512], F32, tag="ps1", name=f"pox{c}")
                          for c in range(2)]
                for c in range(2):
                    nc.tensor.matmul(ps_oxb[c][:],
                                     vg_sb[:, 96 * hl:96 * hl + 96],
                                     pxg[:, 512 * c:512 * c + 512],
                                     start=True, stop=True)

                bl = psm.tile([64, 1024], F32, tag="bl", name="bl")
                bxg = psm.tile([64, 1024], F32, tag="bxg", name="bxg")
                for bank in range(2):
                    sl = slice(512 * bank, 512 * bank + 512)
                    nc.vector.stream_shuffle(bl[0:32, sl], ps_ob[bank][64:96, :],
                                             [0] * 32)
                    nc.vector.stream_shuffle(bl[32:64, sl], ps_ob[bank][64:96, :],
                                             [0] * 32)
                    nc.vector.stream_shuffle(bxg[0:32, sl], ps_oxb[bank][64:96, :],
                                             [0] * 32)
                    nc.vector.stream_shuffle(bxg[32:64, sl], ps_oxb[bank][64:96, :],
                                             [0] * 32)
                cp_o = psm.tile([64, 1024], BF16, tag="cpo", name="cp_o")
                cp_ox = psm.tile([64, 1024], BF16, tag="cpox", name="cp_ox")
                for bank in range(2):
                    sl = slice(512 * bank, 512 * bank + 512)
                    nc.vector.tensor_copy(cp_o[:, sl], ps_ob[bank][0:64, :])
                    nc.scalar.copy(cp_ox[:, sl], ps_oxb[bank][0:64, :])
                nc.vector.reciprocal(bl[:], bl[:])
                nc.vector.reciprocal(bxg[:], bxg[:])
                tmp = psm.tile([64, 1024], F32, tag="tmp", name="tmp")
                tmp2 = psm.tile([64, 1024], F32, tag="tmp2", name="tmp2")
                nc.gpsimd.tensor_mul(tmp[:], cp_o[:], bl[:])
                nc.gpsimd.tensor_mul(tmp2[:], cp_ox[:], bxg[:])
                nc.gpsimd.tensor_add(at_sb[h // 2][r0:r0 + 64, :], tmp[:], tmp2[:])
                nc.scalar.copy(gst[:, 2 * h:2 * h + 2], ps_wv[0:65, :])

            # software-pipelined emission: pass-B q/k projections interleave
            # with pass-A attention head groups (PE executes in program order)
            qk0, qk1 = {}, {}
            emit_qk_proj(0, 0, qk0)
            emit_qk_proj(0, 1, qk0)
            v96_0, vg0 = emit_v_proj(0)
            for h in range(0, 4):
                emit_head(h, qk0, v96_0, vg0)
            emit_qk_proj(1, 0, qk1)
            for h in range(4, 8):
                emit_head(h, qk0, v96_0, vg0)
            emit_qk_proj(1, 1, qk1)
            v96_1, vg1 = emit_v_proj(1)
            for h in range(8, 16):
                emit_head(h, qk1, v96_1, vg1)

            # ================= output projection =================
            # prefetch the first weight tiles before the barrier so their DMAs
            # land during the attention tail
            wot_pre = []
            for m in range(2):
                wotp = pw.tile([128, 1024], BF16, tag="wo", bufs=3,
                               name=f"wot{m}")
                wsrc = bass.AP(wo.tensor, wo[m, 0].offset,
                               [[128, 128], [128 * 128, 8], [1, 128]])
                nc.sync.dma_start(wotp[:], wsrc)
                wot_pre.append(wotp)
            tc.no_sync_barrier()
            for m in range(8):
                ps_op = [pps.tile([128, 512], F32, tag="ps1", name=f"pop{c}")
                         for c in range(2)]
                if m < 2:
                    wot = wot_pre[m]
                else:
                    wot = pw.tile([128, 1024], BF16, tag="wo", bufs=3)
                    wsrc = bass.AP(wo.tensor, wo[m, 0].offset,
                                   [[128, 128], [128 * 128, 8], [1, 128]])
                    nc.sync.dma_start(wot[:], wsrc)
                for f in range(8):
                    for c in range(2):
                        nc.tensor.matmul(ps_op[c][:], wot[:, 128 * f:128 * f + 128],
                                         at_sb[f][:, 512 * c:512 * c + 512],
                                         start=(f == 0), stop=(f == 7))
                for c in range(2):
                    ot = pout.tile([128, 512], F32, tag="ot")
                    nc.scalar.activation(ot[:], ps_op[c][:], AF.Identity,
                                         bias=bo_sb[:, m:m + 1])
                    nc.gpsimd.dma_start(outt[128 * m:128 * (m + 1),
                                             512 * c:512 * c + 512], ot[:])
            nc.sync.dma_start(gstats, gst[:])
            if dbg is not None:
                for f in range(8):
                    nc.sync.dma_start(dbg[128 * f:128 * (f + 1), :], at_sb[f][:])
    return nc


_NC_CACHE = {}
LAST = {}


def get_nc():
    if "nc" not in _NC_CACHE:
        nc = bacc.Bacc("TRN2", target_bir_lowering=False, debug=False, num_devices=8)
        build_kernel(nc)
        nc.compile()
        _NC_CACHE["nc"] = nc
    return _NC_CACHE["nc"]


def make_inputs(x, Wq, Wk, Wv, Wo, bo):
    """Build the 8 per-core input maps (all host-side numpy)."""
    x = np.asarray(x, np.float32)
    Wq = np.asarray(Wq, np.float32)
    Wk = np.asarray(Wk, np.float32)
    Wv = np.asarray(Wv, np.float32)
    Wo = np.asarray(Wo, np.float32)
    bo = np.asarray(bo, np.float32)

    wq_r = np.ascontiguousarray(
        Wq.T.reshape(8, 128, 8, 128).transpose(2, 0, 1, 3))  # [ft, d, 128d, 128f]
    wk_r = np.ascontiguousarray(Wk.T.reshape(8, 128, 8, 128).transpose(2, 0, 1, 3))
    wv_r = np.ascontiguousarray(
        Wv.T.reshape(8, 128, 2, 512).transpose(2, 0, 1, 3))  # [fh, d, 128d, 512f]
    wo_r = np.ascontiguousarray(
        Wo.T.reshape(8, 128, 8, 128).transpose(2, 0, 1, 3)).astype(BF)
    # wo_r[m, f, i, j] must be Wo[128m+j, 128f+i] = Wo.T[128f+i, 128m+j]

    ones = np.ones((128, 1), BF)
    zeros = np.zeros((128, 1), BF)
    in_maps = []
    for core in range(8):
        b, j = divmod(core, 4)
        xs = np.zeros((TOKS, D_MODEL), np.float32)
        xs[0] = x[b, 0]
        xs[1] = x[b, T - 1]
        for w in range(NW):
            gb = 8 * j - 1 + w
            if 0 <= gb < NB:
                xs[2 + 128 * w:2 + 128 * (w + 1)] = x[b, 1 + 128 * gb:1 + 128 * (gb + 1)]
        in_maps.append({
            "xt": np.ascontiguousarray(xs.T),
            "wq": wq_r, "wk": wk_r, "wv": wv_r, "wo": wo_r, "bo": bo,
            "maskl": zeros if j == 0 else ones,
            "maskr": zeros if j == 3 else ones,
        })
    return in_maps


def assemble_output(results, x, Wq, Wk, Wv, Wo, bo):
    x = np.asarray(x, np.float32)
    out = np.empty((B, T, D_MODEL), np.float32)
    for core in range(8):
        b, j = divmod(core, 4)
        out[b, 1 + 1024 * j:1 + 1024 * (j + 1), :] = results[core]["outt"].T

    # global token rows, exact on host
    xg = x[:, [0, T - 1], :]                      # [B, 2, D]
    qg = (xg @ Wq.T).reshape(B, 2, H, DK) * SCALE  # [B, 2, H, DK]
    kg = (xg @ Wk.T).reshape(B, 2, H, DK)
    vg = (xg @ Wv.T).reshape(B, 2, H, DK)
    for b in range(B):
        se = np.zeros((H, 2))
        wvs = np.zeros((H, 2, DK))
        for j in range(4):
            g = results[4 * b + j]["gstats"]  # [65, 32]
            for h in range(H):
                for gi in range(2):
                    se[h, gi] += g[64, 2 * h + gi]
                    wvs[h, gi] += g[0:64, 2 * h + gi]
        # add the global-key terms: scores qg . kg
        sgg = np.einsum("ghd,fhd->hgf", qg[b], kg[b])  # [H, 2g(query), 2f(key)]
        egg = np.exp(sgg)
        num = wvs + np.einsum("hgf,fhd->hgd", egg, vg[b])
        den = se + egg.sum(-1)
        og = num / den[..., None]                  # [H, 2, DK]
        for gi, trow in ((0, 0), (1, T - 1)):
            row = og[:, gi, :].reshape(H * DK)
            out[b, trow] = row @ Wo.T + bo
    return out


def kernel(x, Wq, Wk, Wv, Wo, bo):
    nc = get_nc()
    in_maps = make_inputs(x, Wq, Wk, Wv, Wo, bo)
    res = run_bass_kernel_spmd(nc, in_maps, core_ids=list(range(8)))
    LAST["res"] = res
    results = [{k: np.asarray(v) for k, v in r.items()} for r in res.results]
    return assemble_output(results, x, Wq, Wk, Wv, Wo, bo)


# revision 36
# speedup vs baseline: 1.0050x; 1.0050x over previous
"""BigBird sparse attention kernel for 8 Trainium2 NeuronCores.

Sharding: token-parallel. B=2 batches x 4 chunks of 1024 local tokens each
-> 8 cores. Each core receives a transposed x-slice [D=1024, 1282] whose
columns are [g0, g1, 10 window blocks of 128 tokens] (blocks 8j-1 .. 8j+8,
zero-padded outside [0, 32)). The core computes:
  - q/k projections in transposed layout [f, tok] (fp32r matmuls)
  - v projection in [tok, f] layout
  - 3-block sliding-window attention with scores kept transposed [kt, q]
    (exp'd probabilities feed P.V matmuls directly, denominator rides as a
    ones column in the V stationary)
  - attention of local tokens to the 2 global tokens (separate softmax)
  - flash-style partial stats (sum-exp, weighted V) of the 2 global query
    tokens against the core's local keys -> combined on host
  - output projection + bias for its 1024 local tokens
Host assembles the 8 slices, and computes the 2 global output rows per
batch exactly in numpy from the shipped partials.
"""

import numpy as np
import ml_dtypes

import concourse.bass as bass
import concourse.mybir as mybir
import concourse.tile as tile
from concourse import bacc
from concourse.bass_utils import run_bass_kernel_spmd

F32 = mybir.dt.float32
F32R = mybir.dt.float32r
BF16 = mybir.dt.bfloat16
AF = mybir.ActivationFunctionType
BF = ml_dtypes.bfloat16

D_MODEL = 1024
H = 16
DK = 64
BS = 128
B = 2
T = 4098
NB = 32            # global 128-blocks of local tokens
NW = 10            # window blocks per core (8 local + 2 halo)
TOKS = 2 + NW * BS # x-slice columns
LQ0 = 2 + BS       # first local-q column
SCALE = 1.0 / np.sqrt(DK)

# token chunks for the projection moving dim (all >=256 for fp32r speed)
CHUNKS = [(0, 512), (512, 512), (1024, 258)]


def C(t):
    return 2 + BS * t


# P.V accumulation schedule per psum bank: (t, qstart, nblocks, start, stop)
# bank 0 covers q window-positions 1..4, bank 1 covers 5..8.
PV_SCHED = [
    [(2, 1, 3, True, False), (3, 2, 2, False, False), (3, 4, 1, False, False),
     (0, 1, 1, False, False), (1, 1, 2, False, False), (4, 3, 2, False, False),
     (5, 4, 1, False, True)],
    [(6, 5, 3, True, False), (7, 6, 2, False, False), (7, 8, 1, False, False),
     (4, 5, 1, False, False), (5, 5, 2, False, False), (8, 7, 2, False, False),
     (9, 8, 1, False, True)],
]


def ptcol(t, qpos):
    # column of (window-block t, q window-position qpos) in the pt tensor
    return 384 * t + 128 * (qpos - (t - 1))


def build_kernel(nc):
    xt = nc.dram_tensor("xt", [D_MODEL, TOKS], F32, kind="ExternalInput").ap()
    wq = nc.dram_tensor("wq", [8, 8, 128, 128], F32, kind="ExternalInput").ap()
    wk = nc.dram_tensor("wk", [8, 8, 128, 128], F32, kind="ExternalInput").ap()
    wv = nc.dram_tensor("wv", [2, 8, 128, 512], F32, kind="ExternalInput").ap()
    wo = nc.dram_tensor("wo", [8, 8, 128, 128], BF16, kind="ExternalInput").ap()
    bo = nc.dram_tensor("bo", [D_MODEL], F32, kind="ExternalInput").ap()
    maskl = nc.dram_tensor("maskl", [128, 1], BF16, kind="ExternalInput").ap()
    maskr = nc.dram_tensor("maskr", [128, 1], BF16, kind="ExternalInput").ap()
    outt = nc.dram_tensor("outt", [D_MODEL, 1024], F32, kind="ExternalOutput").ap()
    gstats = nc.dram_tensor("gstats", [65, 32], F32, kind="ExternalOutput").ap()
    import os as _os
    dbg = None
    if _os.environ.get("BB_DEBUG"):
        dbg = nc.dram_tensor("dbg_at", [D_MODEL, 1024], BF16,
                             kind="ExternalOutput").ap()

    with tile.TileContext(nc) as tc:
        with (
            tc.tile_pool(name="pc", bufs=1) as pc,
            tc.tile_pool(name="px", bufs=1) as px,
            tc.tile_pool(name="pqk", bufs=1) as pqk,
            tc.tile_pool(name="pv", bufs=1) as pvp,
            tc.tile_pool(name="pwv", bufs=1) as pwv,
            tc.tile_pool(name="pw", bufs=6) as pw,
            tc.tile_pool(name="pat", bufs=1) as pat,
            tc.tile_pool(name="ppt", bufs=2) as ppt,
            tc.tile_pool(name="psm", bufs=2) as psm,
            tc.tile_pool(name="pout", bufs=2) as pout,
            tc.tile_pool(name="pps", bufs=8, space="PSUM") as pps,
        ):
            # ---- constants ----
            bo_sb = pc.tile([128, 8], F32, tag="bo")
            nc.sync.dma_start(bo_sb[:], bo.rearrange("(t p) -> p t", p=128))
            ml_sb = pc.tile([128, 1], BF16, tag="ml")
            mr_sb = pc.tile([128, 1], BF16, tag="mr")
            nc.sync.dma_start(ml_sb[:], maskl)
            nc.sync.dma_start(mr_sb[:], maskr)
            ones32 = pc.tile([2, 32], BF16, tag="ones32")
            nc.vector.memset(ones32[:], 0.0)
            nc.vector.memset(ones32[:, 0:1], 1.0)
            gst = pc.tile([65, 32], F32, tag="gst")

            # ---- x slice, transposed, resident ----
            xts = []
            for d in range(8):
                xd = px.tile([128, TOKS], F32R, tag=f"xt{d}")
                rows = xt[128 * d:128 * (d + 1), :].bitcast(F32R)
                nc.sync.dma_start(xd[:, 0:512], rows[:, 0:512])
                nc.sync.dma_start(xd[:, 512:TOKS], rows[:, 512:TOKS])
                xts.append(xd)

            at_sb = [pat.tile([128, 1024], BF16, tag=f"at{f}", name=f"at{f}")
                     for f in range(8)]

            def emit_qk_proj(pss, half, qk_tiles):
                for pname, wdram in (("q", wq), ("k", wk)):
                    osbs, psjs = [], []
                    for i2 in range(2):
                        i = 2 * half + i2
                        osb = pqk.tile([128, TOKS], F32R, tag=f"qk{pname}{i}",
                                       name=f"qk{pname}{i}")
                        qk_tiles[(pname, i)] = osb
                        osbs.append(osb)
                        psjs.append([pps.tile([128, cn], F32, tag="ps1",
                                              name=f"pj{i2}_{c}")
                                     for c, (c0, cn) in enumerate(CHUNKS)])
                    for d in range(8):
                        wt = pw.tile([128, 256], F32R, tag="w")
                        ft0 = 4 * pss + 2 * half
                        src = bass.AP(wdram.tensor,
                                      wdram[ft0, d].offset,
                                      [[128, 128], [8 * 128 * 128, 2], [1, 128]])
                        nc.sync.dma_start(wt[:], src.bitcast(F32R))
                        for i2 in range(2):
                            for c, (c0, cn) in enumerate(CHUNKS):
                                nc.tensor.matmul(
                                    psjs[i2][c][:, :cn],
                                    wt[:, 128 * i2:128 * i2 + 128],
                                    xts[d][:, c0:c0 + cn],
                                    start=(d == 0), stop=(d == 7))
                    for i2 in range(2):
                        for c, (c0, cn) in enumerate(CHUNKS):
                            if pname == "q":
                                nc.scalar.mul(osbs[i2][:, c0:c0 + cn],
                                              psjs[i2][c][:, :cn], SCALE)
                            else:
                                nc.scalar.copy(osbs[i2][:, c0:c0 + cn],
                                               psjs[i2][c][:, :cn])

            def emit_v_proj(pss):
                wv_sb = []
                for d in range(8):
                    wvd = pwv.tile([128, 512], F32R, tag=f"wv{d}", name=f"wv{d}")
                    nc.sync.dma_start(wvd[:], wv[pss, d].bitcast(F32R))
                    wv_sb.append(wvd)
                v96 = []
                for tb in range(NW):
                    pv_ps = pps.tile([128, 512], F32, tag="ps1", name="pv_ps")
                    for d in range(8):
                        nc.tensor.matmul(pv_ps[:], xts[d][:, C(tb):C(tb) + 128],
                                         wv_sb[d][:], start=(d == 0), stop=(d == 7))
                    vt = pvp.tile([128, 8 * 96], BF16, tag=f"v96_{tb}",
                                  name=f"v96_{tb}")
                    pstep = vt.ap[0][0]
                    dst = bass.AP(vt.tensor, vt[:].offset,
                                  [[pstep, 128], [96, 8], [1, 64]])
                    src = bass.AP(pv_ps.tensor, pv_ps[:].offset,
                                  [[pv_ps.ap[0][0], 128], [64, 8], [1, 64]])
                    nc.vector.tensor_copy(dst, src)
                    onesap = bass.AP(vt.tensor, vt[:].offset + 64,
                                     [[pstep, 128], [96, 8], [1, 1]])
                    nc.vector.memset(onesap, 1.0)
                    zap = bass.AP(vt.tensor, vt[:].offset + 65,
                                  [[pstep, 128], [96, 8], [1, 31]])
                    nc.vector.memset(zap, 0.0)
                    v96.append(vt)
                pvg = pps.tile([2, 512], F32, tag="ps1", name="pvg")
                for d in range(8):
                    nc.tensor.matmul(pvg[:], xts[d][:, 0:2], wv_sb[d][:],
                                     start=(d == 0), stop=(d == 7))
                vg_sb = pvp.tile([2, 8 * 96], BF16, tag="vg", name="vg")
                gstep = vg_sb.ap[0][0]
                gdst = bass.AP(vg_sb.tensor, vg_sb[:].offset,
                               [[gstep, 2], [96, 8], [1, 64]])
                gsrc = bass.AP(pvg.tensor, pvg[:].offset,
                               [[pvg.ap[0][0], 2], [64, 8], [1, 64]])
                nc.scalar.copy(gdst, gsrc)
                g1 = bass.AP(vg_sb.tensor, vg_sb[:].offset + 64,
                             [[gstep, 2], [96, 8], [1, 1]])
                nc.vector.memset(g1, 1.0)
                g0 = bass.AP(vg_sb.tensor, vg_sb[:].offset + 65,
                             [[gstep, 2], [96, 8], [1, 31]])
                nc.vector.memset(g0, 0.0)
                return v96, vg_sb

            def emit_head(h, qk_tiles, v96, vg_sb):
                hl = h % 8
                r0 = 64 * (hl % 2)
                qh = qk_tiles[("q", hl // 2)][r0:r0 + 64, :]
                kh = qk_tiles[("k", hl // 2)][r0:r0 + 64, :]

                # xg scores + exp first so ACT serves them before the
                # score exps (oxg/wv matmuls then never wait on ACT backlog)
                pxg = psm.tile([2, 1024], BF16, tag="pxg", name="pxg", bufs=3)
                for c in range(2):
                    ps_xg = pps.tile([2, 512], F32, tag="ps1", name="ps_xg")
                    nc.tensor.matmul(ps_xg[:], kh[:, 0:2],
                                     qh[:, LQ0 + 512 * c:LQ0 + 512 * c + 512],
                                     start=True, stop=True)
                    nc.scalar.activation(pxg[:, 512 * c:512 * c + 512], ps_xg[:],
                                         AF.Exp)
                pt = ppt.tile([128, 3840], BF16, tag="pt", name="pt")
                psg = pps.tile([128, 16], F32, tag="ps1", name="psg")
                for t in range(1, 9):
                    nc.tensor.matmul(psg[:, 2 * (t - 1):2 * t],
                                     kh[:, C(t):C(t) + 128], qh[:, 0:2],
                                     start=(t == 1), stop=(t == 8))
                pg = psm.tile([128, 16], BF16, tag="pgsb", name="pg", bufs=3)
                nc.scalar.activation(pg[:], psg[:], AF.Exp)
                for t in range(NW):
                    qlo, qhi = max(t - 1, 1), min(t + 1, 8)
                    n = (qhi - qlo + 1) * 128
                    ps_s = pps.tile([128, 384], F32, tag="ps1", name="ps_s")
                    nc.tensor.matmul(ps_s[:, :n], kh[:, C(t):C(t) + 128],
                                     qh[:, C(qlo):C(qlo) + n],
                                     start=True, stop=True)
                    col = ptcol(t, qlo)
                    nc.scalar.activation(pt[:, col:col + n], ps_s[:, :n], AF.Exp)
                    if t == 0:
                        nc.gpsimd.tensor_mul(pt[:, col:col + n], pt[:, col:col + n],
                                             ml_sb[:].to_broadcast((128, n)))
                    if t == NW - 1:
                        nc.gpsimd.tensor_mul(pt[:, col:col + n], pt[:, col:col + n],
                                             mr_sb[:].to_broadcast((128, n)))
                ps_ob = [pps.tile([96, 512], F32, tag="ps1", name=f"po{bank}")
                         for bank in range(2)]
                for bank in range(2):
                    for (t, qs, nb, st, sp) in PV_SCHED[bank]:
                        c0 = 128 * (qs - 1) - 512 * bank
                        nc.tensor.matmul(
                            ps_ob[bank][:, c0:c0 + 128 * nb],
                            v96[t][:, 96 * hl:96 * hl + 96],
                            pt[:, ptcol(t, qs):ptcol(t, qs) + 128 * nb],
                            start=st, stop=sp)
                ps_wv = pps.tile([96, 2], F32, tag="ps1", name="ps_wv")
                for t in range(1, 9):
                    nc.tensor.matmul(ps_wv[:], v96[t][:, 96 * hl:96 * hl + 96],
                                     pg[:, 2 * (t - 1):2 * t],
                                     start=(t == 1), stop=(t == 8))
                ps_oxb = [pps.tile([96, 512], F32, tag="ps1", name=f"pox{c}")
                          for c in range(2)]
                for c in range(2):
                    nc.tensor.matmul(ps_oxb[c][:],
                                     vg_sb[:, 96 * hl:96 * hl + 96],
                                     pxg[:, 512 * c:512 * c + 512],
                                     start=True, stop=True)

                bl = psm.tile([64, 1024], F32, tag="bl", name="bl")
                bxg = psm.tile([64, 1024], F32, tag="bxg", name="bxg")
                for bank in range(2):
                    sl = slice(512 * bank, 512 * bank + 512)
                    nc.vector.stream_shuffle(bl[0:32, sl], ps_ob[bank][64:96, :],
                                             [0] * 32)
                    nc.vector.stream_shuffle(bl[32:64, sl], ps_ob[bank][64:96, :],
                                             [0] * 32)
                    nc.vector.stream_shuffle(bxg[0:32, sl], ps_oxb[bank][64:96, :],
                                             [0] * 32)
                    nc.vector.stream_shuffle(bxg[32:64, sl], ps_oxb[bank][64:96, :],
                                             [0] * 32)
                cp_o = psm.tile([64, 1024], BF16, tag="cpo", name="cp_o")
                cp_ox = psm.tile([64, 1024], BF16, tag="cpox", name="cp_ox")
                for bank in range(2):
                    sl = slice(512 * bank, 512 * bank + 512)
                    nc.vector.tensor_copy(cp_o[:, sl], ps_ob[bank][0:64, :])
                    nc.scalar.copy(cp_ox[:, sl], ps_oxb[bank][0:64, :])
                nc.vector.reciprocal(bl[:], bl[:])
                nc.vector.reciprocal(bxg[:], bxg[:])
                tmp = psm.tile([64, 1024], F32, tag="tmp", name="tmp")
                tmp2 = psm.tile([64, 1024], F32, tag="tmp2", name="tmp2")
                nc.gpsimd.tensor_mul(tmp[:], cp_o[:], bl[:])
                nc.gpsimd.tensor_mul(tmp2[:], cp_ox[:], bxg[:])
                nc.gpsimd.tensor_add(at_sb[h // 2][r0:r0 + 64, :], tmp[:], tmp2[:])
                nc.scalar.copy(gst[:, 2 * h:2 * h + 2], ps_wv[0:65, :])

            # software-pipelined emission: pass-B q/k projections interleave
            # with pass-A attention head groups (PE executes in program order)
            qk0, qk1 = {}, {}
            emit_qk_proj(0, 0, qk0)
            emit_qk_proj(0, 1, qk0)
            v96_0, vg0 = emit_v_proj(0)
            for h in range(0, 4):
                emit_head(h, qk0, v96_0, vg0)
            emit_qk_proj(1, 0, qk1)
            for h in range(4, 8):
                emit_head(h, qk0, v96_0, vg0)
            emit_qk_proj(1, 1, qk1)
            v96_1, vg1 = emit_v_proj(1)
            for h in range(8, 16):
                emit_head(h, qk1, v96_1, vg1)

            # ================= output projection =================
            # prefetch the first weight tiles before the barrier so their DMAs
            # land during the attention tail
            wot_pre = []
            for m in range(2):
                wotp = pw.tile([128, 1024], BF16, tag="wo", bufs=3,
                               name=f"wot{m}")
                wsrc = bass.AP(wo.tensor, wo[m, 0].offset,
                               [[128, 128], [128 * 128, 8], [1, 128]])
                nc.sync.dma_start(wotp[:], wsrc)
                wot_pre.append(wotp)
            tc.no_sync_barrier()
            for m in range(8):
                ps_op = [pps.tile([128, 512], F32, tag="ps1", name=f"pop{c}")
                         for c in range(2)]
                if m < 2:
                    wot = wot_pre[m]
                else:
                    wot = pw.tile([128, 1024], BF16, tag="wo", bufs=3)
                    wsrc = bass.AP(wo.tensor, wo[m, 0].offset,
                                   [[128, 128], [128 * 128, 8], [1, 128]])
                    nc.sync.dma_start(wot[:], wsrc)
                for f in range(8):
                    for c in range(2):
                        nc.tensor.matmul(ps_op[c][:], wot[:, 128 * f:128 * f + 128],
                                         at_sb[f][:, 512 * c:512 * c + 512],
                                         start=(f == 0), stop=(f == 7))
                for c in range(2):
                    ot = pout.tile([128, 512], F32, tag="ot")
                    nc.scalar.activation(ot[:], ps_op[c][:], AF.Identity,
                                         bias=bo_sb[:, m:m + 1])
                    nc.sync.dma_start(outt[128 * m:128 * (m + 1),
                                           512 * c:512 * c + 512], ot[:])
            nc.sync.dma_start(gstats, gst[:])
            if dbg is not None:
                for f in range(8):
                    nc.sync.dma_start(dbg[128 * f:128 * (f + 1), :], at_sb[f][:])
    return nc


_NC_CACHE = {}
LAST = {}


def get_nc():
    if "nc" not in _NC_CACHE:
        nc = bacc.Bacc("TRN2", target_bir_lowering=False, debug=False, num_devices=8)
        build_kernel(nc)
        nc.compile()
        _NC_CACHE["nc"] = nc
    return _NC_CACHE["nc"]


def make_inputs(x, Wq, Wk, Wv, Wo, bo):
    """Build the 8 per-core input maps (all host-side numpy)."""
    x = np.asarray(x, np.float32)
    Wq = np.asarray(Wq, np.float32)
    Wk = np.asarray(Wk, np.float32)
    Wv = np.asarray(Wv, np.float32)
    Wo = np.asarray(Wo, np.float32)
    bo = np.asarray(bo, np.float32)

    wq_r = np.ascontiguousarray(
        Wq.T.reshape(8, 128, 8, 128).transpose(2, 0, 1, 3))  # [ft, d, 128d, 128f]
    wk_r = np.ascontiguousarray(Wk.T.reshape(8, 128, 8, 128).transpose(2, 0, 1, 3))
    wv_r = np.ascontiguousarray(
        Wv.T.reshape(8, 128, 2, 512).transpose(2, 0, 1, 3))  # [fh, d, 128d, 512f]
    wo_r = np.ascontiguousarray(
        Wo.T.reshape(8, 128, 8, 128).transpose(2, 0, 1, 3)).astype(BF)
    # wo_r[m, f, i, j] must be Wo[128m+j, 128f+i] = Wo.T[128f+i, 128m+j]

    ones = np.ones((128, 1), BF)
    zeros = np.zeros((128, 1), BF)
    in_maps = []
    for core in range(8):
        b, j = divmod(core, 4)
        xs = np.zeros((TOKS, D_MODEL), np.float32)
        xs[0] = x[b, 0]
        xs[1] = x[b, T - 1]
        for w in range(NW):
            gb = 8 * j - 1 + w
            if 0 <= gb < NB:
                xs[2 + 128 * w:2 + 128 * (w + 1)] = x[b, 1 + 128 * gb:1 + 128 * (gb + 1)]
        in_maps.append({
            "xt": np.ascontiguousarray(xs.T),
            "wq": wq_r, "wk": wk_r, "wv": wv_r, "wo": wo_r, "bo": bo,
            "maskl": zeros if j == 0 else ones,
            "maskr": zeros if j == 3 else ones,
        })
    return in_maps


def assemble_output(results, x, Wq, Wk, Wv, Wo, bo):
    x = np.asarray(x, np.float32)
    out = np.empty((B, T, D_MODEL), np.float32)
    for core in range(8):
        b, j = divmod(core, 4)
        out[b, 1 + 1024 * j:1 + 1024 * (j + 1), :] = results[core]["outt"].T

    # global token rows, exact on host
    xg = x[:, [0, T - 1], :]                      # [B, 2, D]
    qg = (xg @ Wq.T).reshape(B, 2, H, DK) * SCALE  # [B, 2, H, DK]
    kg = (xg @ Wk.T).reshape(B, 2, H, DK)
    vg = (xg @ Wv.T).reshape(B, 2, H, DK)
    for b in range(B):
        se = np.zeros((H, 2))
        wvs = np.zeros((H, 2, DK))
        for j in range(4):
            g = results[4 * b + j]["gstats"]  # [65, 32]
            for h in range(H):
                for gi in range(2):
                    se[h, gi] += g[64, 2 * h + gi]
                    wvs[h, gi] += g[0:64, 2 * h + gi]
        # add the global-key terms: scores qg . kg
        sgg = np.einsum("ghd,fhd->hgf", qg[b], kg[b])  # [H, 2g(query), 2f(key)]
        egg = np.exp(sgg)
        num = wvs + np.einsum("hgf,fhd->hgd", egg, vg[b])
        den = se + egg.sum(-1)
        og = num / den[..., None]                  # [H, 2, DK]
        for gi, trow in ((0, 0), (1, T - 1)):
            row = og[:, gi, :].reshape(H * DK)
            out[b, trow] = row @ Wo.T + bo
    return out


def kernel(x, Wq, Wk, Wv, Wo, bo):
    nc = get_nc()
    in_maps = make_inputs(x, Wq, Wk, Wv, Wo, bo)
    res = run_bass_kernel_spmd(nc, in_maps, core_ids=list(range(8)))
    LAST["res"] = res
    results = [{k: np.asarray(v) for k, v in r.items()} for r in res.results]
    return assemble_output(results, x, Wq, Wk, Wv, Wo, bo)
